# revision 1
# baseline (speedup 1.0000x reference)
"""Trainium2 Bass kernel for nn_BinRegularizer (histogram_binning).

Replicates the reference's sequential-f32 segment_sum numerics:
- per-chunk (2048-element) fused compute+reduce passes on DVE + ACT produce
  counts, relu sums, and rne-quantized cumulative sums on dyadic grids
- the host replays the sequential-f32 accumulation trajectory per bin at
  chunk granularity (while the running partial sits in a binade with ulp u,
  each element contributes u*rne(x/u) exactly), then computes the 5 outputs.

Self-contained: hardcodes shapes (4096x16384 f32 weights, alpha[1]),
8 NeuronCores, sharding = contiguous 8M-element blocks per core.
"""
import sys

sys.path.insert(0, "/opt/trn_rl_repo")

import numpy as np

f32 = np.float32

P = 128          # partitions
F = 2048         # free dim per tile = chunk size
NT = 32          # tiles per core
NCORES = 8
CORE_ELEMS = P * F * NT          # 8M
N_TOTAL = CORE_ELEMS * NCORES    # 64M
NCHUNK = NCORES * NT * P         # 32768 chunks of 2048, stream order

# dyadic grids measured on device
US_S = [2.0**-7, 2.0**-6, 2.0**-5, 2.0**-4, 2.0**-3, 2.0**-2]
AV_S = {0: US_S[:5], 1: US_S[:5], 3: US_S}
US_Q = [2.0**-12, 2.0**-11, 2.0**-10, 2.0**-9, 2.0**-8, 2.0**-7, 2.0**-6, 2.0**-5]
AV_Q = {0: [2.0**-9, 2.0**-8, 2.0**-7, 2.0**-6],
        1: [2.0**-10, 2.0**-9, 2.0**-8, 2.0**-7],
        2: [2.0**-12, 2.0**-11, 2.0**-10],
        3: [2.0**-10, 2.0**-9, 2.0**-8, 2.0**-7, 2.0**-6, 2.0**-5]}
# relu-type quantities computed on DVE (stt with zeros) instead of ACT;
# chosen for engine balance
DVE_RELUS = {
    "qLm2@-12", "qL0@-12", "qLp2@-12",
    "qLm2@-11", "qL0@-11", "qLp2@-11",
    "qLm1@-10", "qLm2@-10", "qL0@-10", "qLp2@-10",
    "sL1@-7", "sL2@-7", "sR3@-7",
    "sL1@-6", "sL2@-6",
}

NC_SLOTS = 96

_qneed = {}
for _k, _us in AV_Q.items():
    for _u in _us:
        _qneed.setdefault(int(np.log2(_u)), set()).add(_k)


def MS(u):
    return f32(f32(3.0 * 2.0**22) * f32(u))


def _qz_of(x, u):
    m = MS(u)
    return f32(f32(f32(x) + m) - m)


def _const_values(a):
    """slot-name -> f32 value. Shared vocabulary with the builder."""
    th1 = f32(f32(-1.5) * a)
    th2 = f32(f32(-0.5) * a)
    th3 = f32(f32(0.5) * a)
    tau1 = f32(th1 * th1)
    tau2 = f32(th3 * th3)
    lv0 = f32(f32(-2) * a)
    lv1 = f32(f32(-1) * a)
    lv3 = f32(f32(1) * a)
    vals = {
        "th1": th1, "th2": th2, "th3": th3,
        "nth1": f32(-th1), "nth2": f32(-th2), "nth3": f32(-th3),
        "nm2a": f32(-lv0), "nm1a": f32(-lv1), "np1a": f32(-lv3),
    }
    for u in US_S:
        lg = int(np.log2(u))
        m = MS(u)
        vals[f"st1@{lg}"] = f32(m + _qz_of(th1, u))
        vals[f"st2@{lg}"] = f32(m + _qz_of(th2, u))
        vals[f"st3@{lg}"] = f32(m + _qz_of(th3, u))
        vals[f"nst3@{lg}"] = f32(-vals[f"st3@{lg}"])
    for u in US_Q:
        lg = int(np.log2(u))
        m = MS(u)
        vals[f"qm1@{lg}"] = f32(m - _qz_of(tau1, u))
        vals[f"qm2@{lg}"] = f32(m - _qz_of(tau2, u))
        vals[f"qz0@{lg}"] = m
        vals[f"qp2@{lg}"] = f32(m + _qz_of(tau2, u))
        vals[f"nqp2@{lg}"] = f32(-vals[f"qp2@{lg}"])
    return vals


def _s_names(u):
    return ["sR3"] if u == US_S[-1] else ["sL1", "sL2", "sR3"]


def _q_names(lg):
    ks = _qneed[lg]
    names = []
    if 0 in ks or 1 in ks:
        names.append(("qLm1", f"qm1@{lg}", True))
    if 1 in ks or 2 in ks:
        names.append(("qLm2", f"qm2@{lg}", True))
    if 2 in ks:
        names.append(("qL0", f"qz0@{lg}", True))
        names.append(("qLp2", f"qp2@{lg}", True))
    if 3 in ks:
        names.append(("qRp2", f"qp2@{lg}", False))
    return names


_CACHE = {}


def _build_program():
    import concourse.bacc as bacc
    import concourse.tile as tile
    from concourse import mybir

    AL = mybir.AluOpType
    AF = mybir.ActivationFunctionType
    DT = mybir.dt.float32

    cslot = {}

    def slot(name):
        if name not in cslot:
            cslot[name] = len(cslot)
        return cslot[name]

    # ---- stats layouts ----
    layout_d = {}
    layout_a = {}

    def alloc_stat(name, eng):
        lay = layout_d if eng == "d" else layout_a
        lay[name] = len(lay)

    for nm in ("C1", "C2", "C3", "T1", "E1", "E2", "E3"):
        alloc_stat(nm, "d")
    for nm in ("Bm2a", "B1", "Bm1a", "B2", "B0", "B3", "B1a", "T2"):
        alloc_stat(nm, "a")
    for u in US_S:
        lg = int(np.log2(u))
        for nm in _s_names(u):
            full = f"{nm}@{lg}"
            alloc_stat(full, "d" if full in DVE_RELUS else "a")
    for u in US_Q:
        lg = int(np.log2(u))
        for nm, _cn, _lo in _q_names(lg):
            full = f"{nm}@{lg}"
            alloc_stat(full, "d" if full in DVE_RELUS else "a")
    NQ_D = len(layout_d)
    NQ_A = len(layout_a)

    nc = bacc.Bacc("TRN2", target_bir_lowering=False, debug=False,
                   num_devices=NCORES)
    W = nc.dram_tensor("w", [NT * P, F], DT, kind="ExternalInput")
    CONST = nc.dram_tensor("consts", [P, NC_SLOTS], DT, kind="ExternalInput")
    OUTD = nc.dram_tensor("outd", [P, NQ_D * NT], DT, kind="ExternalOutput")
    OUTA = nc.dram_tensor("outa", [P, NQ_A * NT], DT, kind="ExternalOutput")
    Wv = W[:, :].rearrange("(t p) f -> t p f", p=P)

    with tile.TileContext(nc) as tc:
        with tc.tile_pool(name="wp", bufs=3) as wpool, \
             tc.tile_pool(name="yp", bufs=2) as ypool, \
             tc.tile_pool(name="y2p", bufs=2) as y2pool, \
             tc.tile_pool(name="rp", bufs=4) as rpool, \
             tc.tile_pool(name="zp", bufs=2) as zpool, \
             tc.tile_pool(name="singles", bufs=1) as singles:
            cd = singles.tile([P, NC_SLOTS], DT)
            ca = singles.tile([P, NC_SLOTS], DT)
            zeros = singles.tile([P, F], DT)
            std = singles.tile([P, NQ_D * NT], DT)
            sta = singles.tile([P, NQ_A * NT], DT)
            gd = singles.tile([P, F], DT)
            ga = singles.tile([P, F], DT)

            nc.sync.dma_start(out=cd, in_=CONST[:, :])
            nc.scalar.copy(out=ca, in_=cd)
            nc.vector.memset(zeros, 0.0)

            def cs_d(nm):
                i = slot(nm)
                return cd[:, i:i + 1]

            def cs_a(nm):
                i = slot(nm)
                return ca[:, i:i + 1]

            def st(name, t):
                if name in layout_d:
                    q = layout_d[name]
                    return std[:, q * NT + t:q * NT + t + 1]
                q = layout_a[name]
                return sta[:, q * NT + t:q * NT + t + 1]

            for t in range(NT):
                w = wpool.tile([P, F], DT, tag="w")
                nc.sync.dma_start(out=w, in_=Wv[t])

                # ---- phase A on DVE: counts + T1 ----
                for nm, cn, op in (("C1", "th1", AL.is_gt),
                                   ("C2", "th2", AL.is_ge),
                                   ("C3", "th3", AL.is_gt)):
                    nc.vector.tensor_scalar(
                        out=gd[:, :], in0=w[:, :], scalar1=cs_d(cn),
                        scalar2=None, op0=op, op1=AL.add,
                        accum_out=st(nm, t))
                nc.vector.tensor_scalar(
                    out=gd[:, :], in0=w[:, :], scalar1=0.0,
                    scalar2=None, op0=AL.add, op1=AL.add,
                    accum_out=st("T1", t))

                # ---- phase A on ACT: B relus, T2, |w| ----
                rtiles = {}
                for nm, cn in (("Bm2a", "nm2a"), ("B1", "nth1"),
                               ("Bm1a", "nm1a"), ("B2", "nth2"),
                               ("B0", None), ("B3", "nth3"),
                               ("B1a", "np1a")):
                    if nm in ("B1", "B2", "B3"):
                        rout = rpool.tile([P, F], DT, tag="r")
                        rtiles[nm] = rout
                    else:
                        rout = ga
                    nc.scalar.activation(
                        out=rout[:, :], in_=w[:, :], func=AF.Relu,
                        bias=(0.0 if cn is None else cs_a(cn)), scale=1.0,
                        accum_out=st(nm, t))
                absw = zpool.tile([P, F], DT, tag="absw")
                nc.scalar.activation(out=absw[:, :], in_=w[:, :], func=AF.Abs,
                                     bias=0.0, scale=1.0)
                nc.scalar.activation(out=ga[:, :], in_=w[:, :], func=AF.Square,
                                     bias=0.0, scale=1.0, accum_out=st("T2", t))

                # ---- E sums on DVE ----
                for j, nm in enumerate(("B1", "B2", "B3")):
                    nc.vector.scalar_tensor_tensor(
                        out=gd[:, :], in0=rtiles[nm][:, :], scalar=1.0,
                        in1=rtiles[nm][:, :], op0=AL.mult, op1=AL.mult,
                        accum_out=st(f"E{j + 1}", t))

                # ---- signed square z = w * |w| ----
                z = zpool.tile([P, F], DT, tag="z")
                nc.vector.tensor_mul(out=z[:, :], in0=w[:, :], in1=absw[:, :])

                def emit_cum(full, src_tile, cn, lower, t):
                    if full in DVE_RELUS:
                        nc.vector.scalar_tensor_tensor(
                            out=gd[:, :], in0=src_tile[:, :], scalar=cs_d(cn),
                            in1=zeros[:, :], op0=AL.subtract,
                            op1=(AL.min if lower else AL.max),
                            accum_out=st(full, t))
                    elif lower:
                        nc.scalar.activation(
                            out=ga[:, :], in_=src_tile[:, :], func=AF.Relu,
                            bias=cs_a(cn), scale=-1.0, accum_out=st(full, t))
                    else:
                        nc.scalar.activation(
                            out=ga[:, :], in_=src_tile[:, :], func=AF.Relu,
                            bias=cs_a("n" + cn), scale=1.0,
                            accum_out=st(full, t))

                # ---- s-side quantized passes ----
                for u in US_S:
                    lg = int(np.log2(u))
                    y = ypool.tile([P, F], DT, tag="y")
                    nc.vector.tensor_scalar(
                        out=y[:, :], in0=w[:, :], scalar1=float(MS(u)),
                        scalar2=None, op0=AL.add)
                    for nm in _s_names(u):
                        cn = {"sL1": f"st1@{lg}", "sL2": f"st2@{lg}",
                              "sR3": f"st3@{lg}"}[nm]
                        emit_cum(f"{nm}@{lg}", y, cn, nm.startswith("sL"), t)

                # ---- sq-side quantized passes (z-space) ----
                for u in US_Q:
                    lg = int(np.log2(u))
                    y2 = y2pool.tile([P, F], DT, tag="y2")
                    nc.vector.tensor_scalar(
                        out=y2[:, :], in0=z[:, :], scalar1=float(MS(u)),
                        scalar2=None, op0=AL.add)
                    for nm, cn, lower in _q_names(lg):
                        emit_cum(f"{nm}@{lg}", y2, cn, lower, t)

            nc.sync.dma_start(out=OUTD[:, :], in_=std)
            nc.sync.dma_start(out=OUTA[:, :], in_=sta)

    assert len(cslot) <= NC_SLOTS, len(cslot)
    nc.compile()
    return nc, cslot, layout_d, layout_a, NQ_D, NQ_A


def _get_program():
    if "prog" not in _CACHE:
        _CACHE["prog"] = _build_program()
    return _CACHE["prog"]


def kernel(weights, alpha):
    from concourse.bass_utils import run_bass_kernel_spmd

    w_full = np.ascontiguousarray(weights, dtype=np.float32).reshape(-1)
    a = f32(np.asarray(alpha, dtype=np.float32).reshape(-1)[0])
    assert w_full.size == N_TOTAL

    nc, cslot, layout_d, layout_a, NQ_D, NQ_A = _get_program()

    vals = _const_values(a)
    cvals = np.zeros(NC_SLOTS, f32)
    for nm, i in cslot.items():
        cvals[i] = vals[nm]
    consts_np = np.ascontiguousarray(np.broadcast_to(cvals, (P, NC_SLOTS)))

    in_maps = []
    for c in range(NCORES):
        shard = w_full[c * CORE_ELEMS:(c + 1) * CORE_ELEMS].reshape(NT * P, F)
        in_maps.append({"w": shard, "consts": consts_np})

    res = run_bass_kernel_spmd(nc, in_maps, core_ids=list(range(NCORES)))

    tabs = {}
    for lay, key, nq in ((layout_d, "outd", NQ_D), (layout_a, "outa", NQ_A)):
        for qname, qi in lay.items():
            arr = np.empty(NCHUNK, np.float64)
            for c in range(NCORES):
                block = res.results[c][key].reshape(P, nq, NT)
                arr[c * NT * P:(c + 1) * NT * P] = block[:, qi, :].T.reshape(-1)
            tabs[qname] = arr

    return _finish(tabs, a)


def _finish(dev, a):
    th1 = f32(f32(-1.5) * a)
    th2 = f32(f32(-0.5) * a)
    th3 = f32(f32(0.5) * a)
    tau1 = f32(th1 * th1)
    tau2 = f32(th3 * th3)
    lv = [f32(f32(-2) * a), f32(f32(-1) * a), f32(0.0), f32(f32(1) * a)]

    nvec = np.full(NCHUNK, float(F))
    C1, C2, C3 = dev["C1"], dev["C2"], dev["C3"]

    S_gt1 = dev["B1"] + float(th1) * C1
    S_ge2 = dev["B2"] + float(th2) * C2
    S_gt3 = dev["B3"] + float(th3) * C3
    s_ab = np.stack([dev["T1"] - S_gt1, S_gt1 - S_ge2, S_ge2 - S_gt3, S_gt3], 1)
    Q_gt1 = dev["E1"] + 2 * float(th1) * dev["B1"] + float(th1)**2 * C1
    Q_ge2 = dev["E2"] + 2 * float(th2) * dev["B2"] + float(th2)**2 * C2
    Q_gt3 = dev["E3"] + 2 * float(th3) * dev["B3"] + float(th3)**2 * C3
    sq_ab = np.stack([dev["T2"] - Q_gt1, Q_gt1 - Q_ge2, Q_ge2 - Q_gt3, Q_gt3], 1)
    cnt_ab = np.stack([nvec - C1, C1 - C2, C2 - C3, C3], 1)

    def getL(name):
        v = dev[name]
        return -v if name in DVE_RELUS else v

    sig_s = {}
    for u in US_S:
        lg = int(np.log2(u))
        qt1 = float(_qz_of(th1, u))
        qt2 = float(_qz_of(th2, u))
        qt3 = float(_qz_of(th3, u))
        G3 = dev[f"sR3@{lg}"] + qt3 * C3
        if u == US_S[-1]:
            zz = np.zeros(NCHUNK)
            sig_s[u] = np.stack([zz, zz, zz, G3], 1)
        else:
            F1 = qt1 * (nvec - C1) - getL(f"sL1@{lg}")
            F2p = qt2 * (nvec - C2) - getL(f"sL2@{lg}")
            sig_s[u] = np.stack([F1, F2p - F1, np.zeros(NCHUNK), G3], 1)

    sig_q = {}
    for u in US_Q:
        lg = int(np.log2(u))
        ks = _qneed[lg]
        qm1 = float(-_qz_of(tau1, u))
        qm2 = float(-_qz_of(tau2, u))
        qp2 = float(_qz_of(tau2, u))
        cols = [np.zeros(NCHUNK) for _ in range(4)]
        Sm1 = Sm2 = S0 = Sp2 = None
        if 0 in ks or 1 in ks:
            Sm1 = qm1 * (nvec - C1) - getL(f"qLm1@{lg}")
        if 1 in ks or 2 in ks:
            Sm2 = qm2 * (nvec - C2) - getL(f"qLm2@{lg}")
        if 2 in ks:
            S0 = -getL(f"qL0@{lg}")
            Sp2 = qp2 * (nvec - C3) - getL(f"qLp2@{lg}")
        if 0 in ks:
            cols[0] = -Sm1
        if 1 in ks:
            cols[1] = -(Sm2 - Sm1)
        if 2 in ks:
            cols[2] = (Sp2 - S0) - (S0 - Sm2)
        if 3 in ks:
            cols[3] = dev[f"qRp2@{lg}"] + qp2 * C3
        sig_q[u] = np.stack(cols, 1)

    def replay(k, tabs, fallback, avail):
        avail = sorted(avail)
        Pp = 0.0
        umin = avail[0] if avail else None
        for c in range(NCHUNK):
            ap = abs(Pp)
            u = 0.0 if ap == 0.0 else 2.0 ** (np.floor(np.log2(ap)) - 23)
            if umin is None or u < umin:
                Pp += fallback[c, k]
            else:
                uu = None
                for x in reversed(avail):
                    if x <= u:
                        uu = x
                        break
                if uu is None:
                    uu = umin
                Pp += tabs[uu][c, k]
        return Pp

    s_rep = np.array([replay(k, sig_s, s_ab, AV_S.get(k, [])) for k in range(4)])
    sq_rep = np.array([replay(k, sig_q, sq_ab, AV_Q[k]) for k in range(4)])

    c_rep = np.minimum(cnt_ab.sum(0), 2.0**24)
    levels = np.array(lv, np.float64)
    safe = np.maximum(c_rep, 1.0)
    mean = s_rep / safe
    var = sq_rep / safe - mean * mean
    total_mse = np.sum(np.where(c_rep > 0, (mean - levels) ** 2, 0.0))
    total_var = np.sum(np.where(c_rep >= 2, var, 0.0))
    loss = total_mse + total_var

    N = float(N_TOTAL)
    sum_absd = (-dev["T1"].sum() - 2 * float(a) * N
                + 2 * (dev["Bm2a"].sum() - dev["B1"].sum() + dev["Bm1a"].sum()
                       - dev["B2"].sum() + dev["B0"].sum() - dev["B3"].sum()
                       + dev["B1a"].sum()))
    s_ex = s_ab.sum(0)
    c_ex = cnt_ab.sum(0)
    sum_w_wq = sum(float(lv[k]) * s_ex[k] for k in range(4))
    sum_wq2 = sum(float(lv[k])**2 * c_ex[k] for k in range(4))
    sum_d2 = dev["T2"].sum() - 2 * sum_w_wq + sum_wq2

    return np.array([loss, total_mse, total_var, sum_d2 / N, sum_absd / N],
                    np.float32)



# revision 16
# speedup vs baseline: 5.5713x; 5.5713x over previous
"""Trainium2 Bass kernel for nn_BinRegularizer (histogram_binning) — v5.

Supertiles of [128, 8192] (chunk = 8192 elems) to amortize per-instruction
overhead (~1-3us/op measured).  Engine-balanced op set per supertile:

  DVE: conv w/a (2 half-ops, fp32->fp16), y=rne(ws), yc=clip(y,-2,1),
       d=ws-yc (accum D), C1,C2 counts (is_gt on yc), T2=sum ws^2 (stt),
       r3=relu(ws-0.5) (stt-zeros, accum R3), P3=(ws-0.5)*r3 (stt)
  ACT: C3 via Sign(yc-0.5) accum, r1/r2=relu(ws+1.5/0.5) (accum R1/R2),
       P1=Square(r1), P2=Square(r2), A=sum|d| via Abs(d)

Host: per-chunk per-bin count/sum/sumsq from slot differences, then model
the reference's sequential-f32 segment_sum (count clip at 2^24 + beta(u)
quantization-loss replay at chunk granularity; beta from strided-8 sample).
Offline validation: max rel err ~6e-3 (gate 2e-2).

Sharding: 8 cores, contiguous 8M-element blocks per core.  Self-contained.
"""
import sys

sys.path.insert(0, "/opt/trn_rl_repo")

import numpy as np

f32 = np.float32

P = 128
F = 8192
NT = 8
NCORES = 8
CORE_ELEMS = P * F * NT
N_TOTAL = CORE_ELEMS * NCORES
NCHUNK = NCORES * NT * P          # 8192 chunks of 8192 elems, stream order

STATS = ["D", "A", "C1", "C2", "C3s", "R1", "M2", "M3",
         "P1", "P2", "P3", "T2"]
NS = len(STATS)
MAGIC = 12582912.0  # 1.5 * 2^23: fp32 rne-to-integer shift

_CACHE = {}


def _build_program():
    import concourse.bacc as bacc
    import concourse.tile as tile
    from concourse import mybir

    AL = mybir.AluOpType
    AF = mybir.ActivationFunctionType
    DT = mybir.dt.float32
    HF = mybir.dt.float16
    H = F // 2

    nc = bacc.Bacc("TRN2", target_bir_lowering=False, debug=False,
                   num_devices=NCORES)
    W = nc.dram_tensor("w", [NT * P, F], DT, kind="ExternalInput")
    CONST = nc.dram_tensor("consts", [P, 8], DT, kind="ExternalInput")
    OUT = nc.dram_tensor("stats", [P, NS * NT], DT, kind="ExternalOutput")
    Wv = W[:, :].rearrange("(t p) f -> t p f", p=P)

    Q = F // 4
    with tile.TileContext(nc) as tc:
        with tc.tile_pool(name="wp", bufs=2) as wpool, \
             tc.tile_pool(name="wbp", bufs=2) as wbpool, \
             tc.tile_pool(name="ycp", bufs=1) as ycpool, \
             tc.tile_pool(name="dp", bufs=1) as dpool, \
             tc.tile_pool(name="mp", bufs=2) as mpool, \
             tc.tile_pool(name="r1p", bufs=1) as r1pool, \
             tc.tile_pool(name="singles", bufs=1) as singles:
            cd = singles.tile([P, 8], DT)
            st = singles.tile([P, NS * NT], DT)
            g16 = singles.tile([P, F], HF)   # DVE garbage / y scratch
            ga = singles.tile([P, F], HF)    # ACT garbage

            nc.sync.dma_start(out=cd, in_=CONST[:, :])
            inv_a = cd[:, 0:1]
            zero = cd[:, 1:2]
            c15 = cd[:, 2:3]
            c05 = cd[:, 3:4]
            cm05 = cd[:, 4:5]

            def slot(name, t):
                q = STATS.index(name)
                return st[:, q * NT + t:q * NT + t + 1]

            for t in range(NT):
                # load + convert in quarters (1MB DMAs)
                wb = wbpool.tile([P, F], HF, tag="wb")
                for h in range(4):
                    w = wpool.tile([P, Q], DT, tag="w")
                    nc.sync.dma_start(out=w, in_=Wv[t][:, h * Q:(h + 1) * Q])
                    nc.vector.tensor_scalar(
                        out=wb[:, h * Q:(h + 1) * Q], in0=w[:, :],
                        scalar1=inv_a, scalar2=None, op0=AL.mult)

                # y (into g16 scratch), yc, d
                nc.vector.tensor_scalar(
                    out=g16[:, :], in0=wb[:, :], scalar1=MAGIC, scalar2=MAGIC,
                    op0=AL.add, op1=AL.subtract)
                yc = ycpool.tile([P, F], HF, tag="yc")
                nc.vector.tensor_scalar(
                    out=yc[:, :], in0=g16[:, :], scalar1=1.0, scalar2=-2.0,
                    op0=AL.min, op1=AL.max)
                dt_ = dpool.tile([P, F], HF, tag="d")
                nc.vector.scalar_tensor_tensor(
                    out=dt_[:, :], in0=yc[:, :], scalar=-1.0, in1=wb[:, :],
                    op0=AL.mult, op1=AL.add, accum_out=slot("D", t))

                # counts: C1, C2 on DVE; C3 on ACT via Sign(yc - 0.5)
                for nm, th in (("C1", -1.5), ("C2", -0.5)):
                    nc.vector.tensor_scalar(
                        out=g16[:, :], in0=yc[:, :], scalar1=th, scalar2=None,
                        op0=AL.is_gt, op1=AL.add, accum_out=slot(nm, t))
                nc.scalar.activation(
                    out=ga[:, :], in_=yc[:, :], func=AF.Sign,
                    bias=cm05, scale=1.0, accum_out=slot("C3s", t))

                # A = sum |d| on ACT
                nc.scalar.activation(
                    out=ga[:, :], in_=dt_[:, :], func=AF.Abs,
                    bias=zero, scale=1.0, accum_out=slot("A", t))

                # r1 = relu(ws+1.5) on ACT (tile + accum R1)
                r1 = r1pool.tile([P, F], HF, tag="r1")
                nc.scalar.activation(
                    out=r1[:, :], in_=wb[:, :], func=AF.Relu,
                    bias=c15, scale=1.0, accum_out=slot("R1", t))
                # m2 = max(ws, -0.5), m3 = max(ws, 0.5) on DVE
                # (accum = b*N + relu-sum)
                m2 = mpool.tile([P, F], HF, tag="m2")
                nc.vector.tensor_scalar(
                    out=m2[:, :], in0=wb[:, :], scalar1=-0.5, scalar2=None,
                    op0=AL.max, op1=AL.add, accum_out=slot("M2", t))
                m3 = mpool.tile([P, F], HF, tag="m3")
                nc.vector.tensor_scalar(
                    out=m3[:, :], in0=wb[:, :], scalar1=0.5, scalar2=None,
                    op0=AL.max, op1=AL.add, accum_out=slot("M3", t))

                # relu^2 sums on ACT: Square(r1), Square(m2+0.5), Square(m3-0.5)
                nc.scalar.activation(
                    out=ga[:, :], in_=r1[:, :], func=AF.Square,
                    bias=zero, scale=1.0, accum_out=slot("P1", t))
                nc.scalar.activation(
                    out=ga[:, :], in_=m2[:, :], func=AF.Square,
                    bias=c05, scale=1.0, accum_out=slot("P2", t))
                nc.scalar.activation(
                    out=ga[:, :], in_=m3[:, :], func=AF.Square,
                    bias=cm05, scale=1.0, accum_out=slot("P3", t))

                # T2 = sum ws^2 on DVE stt
                nc.vector.scalar_tensor_tensor(
                    out=g16[:, :], in0=wb[:, :], scalar=0.0, op0=AL.add,
                    in1=wb[:, :], op1=AL.mult, accum_out=slot("T2", t))

            nc.sync.dma_start(out=OUT[:, :], in_=st)

    nc.compile()
    return nc


def _get_program():
    if "prog" not in _CACHE:
        _CACHE["prog"] = _build_program()
    return _CACHE["prog"]


def _consts_np(a):
    cvals = np.array([f32(1.0) / a, 0.0, 1.5, 0.5, -0.5, 0.0, 0.0, 0.0], f32)
    return np.ascontiguousarray(np.broadcast_to(cvals, (P, 8)))


def kernel(weights, alpha):
    from concourse.bass_utils import run_bass_kernel_spmd

    w_full = np.ascontiguousarray(weights, dtype=np.float32).reshape(-1)
    a = f32(np.asarray(alpha, dtype=np.float32).reshape(-1)[0])
    assert w_full.size == N_TOTAL

    nc = _get_program()
    consts_np = _consts_np(a)

    in_maps = []
    for c in range(NCORES):
        shard = w_full[c * CORE_ELEMS:(c + 1) * CORE_ELEMS].reshape(NT * P, F)
        in_maps.append({"w": shard, "consts": consts_np})

    res = run_bass_kernel_spmd(nc, in_maps, core_ids=list(range(NCORES)))

    dev = {}
    for qi, nm in enumerate(STATS):
        arr = np.empty(NCHUNK, np.float64)
        for c in range(NCORES):
            block = res.results[c]["stats"].reshape(P, NS, NT)
            arr[c * NT * P:(c + 1) * NT * P] = block[:, qi, :].T.reshape(-1)
        dev[nm] = arr

    return _finish(dev, a, w_full)


def _finish(dev, a, w_full):
    N = float(N_TOTAL)
    a = float(a)
    nvec = np.full(NCHUNK, float(F))
    C1, C2 = dev["C1"], dev["C2"]
    C3 = 0.5 * (dev["C3s"] + nvec)
    T1 = dev["D"] + C1 + C2 + C3 - 2.0 * nvec
    R1 = dev["R1"]
    R2 = dev["M2"] + 0.5 * nvec   # sum max(ws,-0.5) = -0.5*n + relu-sum
    R3 = dev["M3"] - 0.5 * nvec
    S1 = R1 - 1.5 * C1
    S2 = R2 - 0.5 * C2
    S3 = R3 + 0.5 * C3
    Q1 = dev["P1"] - 3.0 * R1 + 2.25 * C1
    Q2 = dev["P2"] - 1.0 * R2 + 0.25 * C2
    Q3 = dev["P3"] + 1.0 * R3 + 0.25 * C3
    c_ch = np.stack([nvec - C1, C1 - C2, C2 - C3, C3], 1)
    s_ch = a * np.stack([T1 - S1, S1 - S2, S2 - S3, S3], 1)
    q_ch = a * a * np.stack([dev["T2"] - Q1, Q1 - Q2, Q2 - Q3, Q3], 1)

    # ---- model of the reference's sequential-f32 segment_sum ----
    samp = w_full[::8].astype(np.float64)
    bins_s = np.round(np.clip(w_full[::8].astype(np.float32) / f32(a),
                              -2, 1)).astype(np.int64) + 2
    bin_vals = [samp[bins_s == k] for k in range(4)]
    beta_cache = {}

    def beta(kind, k, u):
        key = (kind, k, int(np.log2(u)))
        if key not in beta_cache:
            v = bin_vals[k]
            v = v * v if kind == "q" else v
            sv = v.sum()
            beta_cache[key] = (u * np.round(v / u)).sum() / sv if sv != 0 else 1.0
        return beta_cache[key]

    def replay(kind, k, deltas):
        nz = deltas[deltas != 0]
        scale = np.median(np.abs(nz)) / F if nz.size else 1.0
        Pp = 0.0
        for m in range(NCHUNK):
            ap = abs(Pp)
            if ap == 0.0:
                Pp += deltas[m]
                continue
            u = 2.0 ** (np.floor(np.log2(ap)) - 23)
            if u < 1e-3 * scale:
                Pp += deltas[m]
            else:
                Pp += beta(kind, k, u) * deltas[m]
        return Pp

    c_tot = c_ch.sum(0)
    c_f32 = np.minimum(c_tot, 2.0 ** 24)
    s_f32 = np.array([replay("s", k, s_ch[:, k]) for k in range(4)])
    q_f32 = np.array([replay("q", k, q_ch[:, k]) for k in range(4)])

    L = np.array([-2.0, -1.0, 0.0, 1.0]) * a
    safe = np.maximum(c_f32, 1.0)
    mean = s_f32 / safe
    var = q_f32 / safe - mean * mean
    total_mse = np.where(c_tot > 0, (mean - L) ** 2, 0.0).sum()
    total_var = np.where(c_tot >= 2, var, 0.0).sum()
    loss = total_mse + total_var

    s_ex = s_ch.sum(0)
    q_ex = q_ch.sum(0)
    sum_d2 = (q_ex - 2 * L * s_ex + L * L * c_tot).sum()
    mean_dist = a * dev["A"].sum() / N

    return np.array([loss, total_mse, total_var, sum_d2 / N, mean_dist],
                    np.float32)


# revision 18
# speedup vs baseline: 6.7361x; 1.2091x over previous
"""Trainium2 Bass kernel for nn_BinRegularizer (histogram_binning) — v5.

Supertiles of [128, 8192] (chunk = 8192 elems) to amortize per-instruction
overhead (~1-3us/op measured).  Engine-balanced op set per supertile:

  DVE: conv w/a (2 half-ops, fp32->fp16), y=rne(ws), yc=clip(y,-2,1),
       d=ws-yc (accum D), C1,C2 counts (is_gt on yc), T2=sum ws^2 (stt),
       r3=relu(ws-0.5) (stt-zeros, accum R3), P3=(ws-0.5)*r3 (stt)
  ACT: C3 via Sign(yc-0.5) accum, r1/r2=relu(ws+1.5/0.5) (accum R1/R2),
       P1=Square(r1), P2=Square(r2), A=sum|d| via Abs(d)

Host: per-chunk per-bin count/sum/sumsq from slot differences, then model
the reference's sequential-f32 segment_sum (count clip at 2^24 + beta(u)
quantization-loss replay at chunk granularity; beta from strided-8 sample).
Offline validation: max rel err ~6e-3 (gate 2e-2).

Sharding: 8 cores, contiguous 8M-element blocks per core.  Self-contained.
"""
import sys

sys.path.insert(0, "/opt/trn_rl_repo")

import numpy as np

f32 = np.float32

P = 128
F = 8192
NT = 8
NCORES = 8
CORE_ELEMS = P * F * NT
N_TOTAL = CORE_ELEMS * NCORES
NCHUNK = NCORES * NT * P          # 8192 chunks of 8192 elems, stream order

STATS = ["D", "A", "C1", "C2", "C3s", "R1", "M2", "M3",
         "P1", "P2", "P3", "T2"]
NS = len(STATS)
MAGIC = 12582912.0  # 1.5 * 2^23: fp32 rne-to-integer shift

_CACHE = {}


def _build_program():
    import concourse.bacc as bacc
    import concourse.tile as tile
    from concourse import mybir

    AL = mybir.AluOpType
    AF = mybir.ActivationFunctionType
    DT = mybir.dt.float32
    HF = mybir.dt.float16
    H = F // 2

    nc = bacc.Bacc("TRN2", target_bir_lowering=False, debug=False,
                   num_devices=NCORES)
    W = nc.dram_tensor("w", [NT * P, F], DT, kind="ExternalInput")
    CONST = nc.dram_tensor("consts", [P, 8], DT, kind="ExternalInput")
    OUT = nc.dram_tensor("stats", [P, NS * NT], DT, kind="ExternalOutput")
    Wv = W[:, :].rearrange("(t p) f -> t p f", p=P)

    Q = F // 4
    with tile.TileContext(nc) as tc:
        with tc.tile_pool(name="wp", bufs=3) as wpool, \
             tc.tile_pool(name="wbp", bufs=2) as wbpool, \
             tc.tile_pool(name="ycp", bufs=1) as ycpool, \
             tc.tile_pool(name="dp", bufs=1) as dpool, \
             tc.tile_pool(name="mp", bufs=2) as mpool, \
             tc.tile_pool(name="r1p", bufs=1) as r1pool, \
             tc.tile_pool(name="singles", bufs=1) as singles:
            cd = singles.tile([P, 8], DT)
            st = singles.tile([P, NS * NT], DT)
            g16 = singles.tile([P, F], HF)   # DVE garbage / y scratch
            ga = singles.tile([P, F], HF)    # ACT garbage

            nc.sync.dma_start(out=cd, in_=CONST[:, :])
            inv_a = cd[:, 0:1]
            zero = cd[:, 1:2]
            c15 = cd[:, 2:3]
            c05 = cd[:, 3:4]
            cm05 = cd[:, 4:5]

            def slot(name, t):
                q = STATS.index(name)
                return st[:, q * NT + t:q * NT + t + 1]

            for t in range(NT):
                # load + convert in quarters (1MB DMAs)
                wb = wbpool.tile([P, F], HF, tag="wb")
                for h in range(4):
                    w = wpool.tile([P, Q], DT, tag="w")
                    nc.sync.dma_start(out=w, in_=Wv[t][:, h * Q:(h + 1) * Q])
                    nc.vector.tensor_scalar(
                        out=wb[:, h * Q:(h + 1) * Q], in0=w[:, :],
                        scalar1=inv_a, scalar2=None, op0=AL.mult)

                # y (into g16 scratch), yc, d
                nc.vector.tensor_scalar(
                    out=g16[:, :], in0=wb[:, :], scalar1=MAGIC, scalar2=MAGIC,
                    op0=AL.add, op1=AL.subtract)
                yc = ycpool.tile([P, F], HF, tag="yc")
                nc.vector.tensor_scalar(
                    out=yc[:, :], in0=g16[:, :], scalar1=1.0, scalar2=-2.0,
                    op0=AL.min, op1=AL.max)
                dt_ = dpool.tile([P, F], HF, tag="d")
                nc.vector.scalar_tensor_tensor(
                    out=dt_[:, :], in0=yc[:, :], scalar=-1.0, in1=wb[:, :],
                    op0=AL.mult, op1=AL.add, accum_out=slot("D", t))

                # r1 = relu(ws+1.5) on ACT (tile + accum R1) - needs only wb
                r1 = r1pool.tile([P, F], HF, tag="r1")
                nc.scalar.activation(
                    out=r1[:, :], in_=wb[:, :], func=AF.Relu,
                    bias=c15, scale=1.0, accum_out=slot("R1", t))

                # m2 = max(ws, -0.5), m3 = max(ws, 0.5) on DVE
                # (accum = b*N + relu-sum)
                m2 = mpool.tile([P, F], HF, tag="m2")
                nc.vector.tensor_scalar(
                    out=m2[:, :], in0=wb[:, :], scalar1=-0.5, scalar2=None,
                    op0=AL.max, op1=AL.add, accum_out=slot("M2", t))
                m3 = mpool.tile([P, F], HF, tag="m3")
                nc.vector.tensor_scalar(
                    out=m3[:, :], in0=wb[:, :], scalar1=0.5, scalar2=None,
                    op0=AL.max, op1=AL.add, accum_out=slot("M3", t))

                # counts: C1, C2 on DVE; C3 on ACT via Sign(yc - 0.5)
                nc.scalar.activation(
                    out=ga[:, :], in_=yc[:, :], func=AF.Sign,
                    bias=cm05, scale=1.0, accum_out=slot("C3s", t))
                for nm, th in (("C1", -1.5), ("C2", -0.5)):
                    nc.vector.tensor_scalar(
                        out=g16[:, :], in0=yc[:, :], scalar1=th, scalar2=None,
                        op0=AL.is_gt, op1=AL.add, accum_out=slot(nm, t))

                # A = sum |d| on ACT
                nc.scalar.activation(
                    out=ga[:, :], in_=dt_[:, :], func=AF.Abs,
                    bias=zero, scale=1.0, accum_out=slot("A", t))

                # relu^2 sums on ACT: Square(r1), Square(m2+0.5), Square(m3-0.5)
                nc.scalar.activation(
                    out=ga[:, :], in_=m2[:, :], func=AF.Square,
                    bias=c05, scale=1.0, accum_out=slot("P2", t))
                nc.scalar.activation(
                    out=ga[:, :], in_=m3[:, :], func=AF.Square,
                    bias=cm05, scale=1.0, accum_out=slot("P3", t))
                nc.scalar.activation(
                    out=ga[:, :], in_=r1[:, :], func=AF.Square,
                    bias=zero, scale=1.0, accum_out=slot("P1", t))

                # T2 = sum ws^2 on DVE stt
                nc.vector.scalar_tensor_tensor(
                    out=g16[:, :], in0=wb[:, :], scalar=0.0, op0=AL.add,
                    in1=wb[:, :], op1=AL.mult, accum_out=slot("T2", t))

            nc.sync.dma_start(out=OUT[:, :], in_=st)

    nc.compile()
    return nc


def _get_program():
    if "prog" not in _CACHE:
        _CACHE["prog"] = _build_program()
    return _CACHE["prog"]


def _consts_np(a):
    cvals = np.array([f32(1.0) / a, 0.0, 1.5, 0.5, -0.5, 0.0, 0.0, 0.0], f32)
    return np.ascontiguousarray(np.broadcast_to(cvals, (P, 8)))


def kernel(weights, alpha):
    from concourse.bass_utils import run_bass_kernel_spmd

    w_full = np.ascontiguousarray(weights, dtype=np.float32).reshape(-1)
    a = f32(np.asarray(alpha, dtype=np.float32).reshape(-1)[0])
    assert w_full.size == N_TOTAL

    nc = _get_program()
    consts_np = _consts_np(a)

    in_maps = []
    for c in range(NCORES):
        shard = w_full[c * CORE_ELEMS:(c + 1) * CORE_ELEMS].reshape(NT * P, F)
        in_maps.append({"w": shard, "consts": consts_np})

    res = run_bass_kernel_spmd(nc, in_maps, core_ids=list(range(NCORES)))

    dev = {}
    for qi, nm in enumerate(STATS):
        arr = np.empty(NCHUNK, np.float64)
        for c in range(NCORES):
            block = res.results[c]["stats"].reshape(P, NS, NT)
            arr[c * NT * P:(c + 1) * NT * P] = block[:, qi, :].T.reshape(-1)
        dev[nm] = arr

    return _finish(dev, a, w_full)


def _finish(dev, a, w_full):
    N = float(N_TOTAL)
    a = float(a)
    nvec = np.full(NCHUNK, float(F))
    C1, C2 = dev["C1"], dev["C2"]
    C3 = 0.5 * (dev["C3s"] + nvec)
    T1 = dev["D"] + C1 + C2 + C3 - 2.0 * nvec
    R1 = dev["R1"]
    R2 = dev["M2"] + 0.5 * nvec   # sum max(ws,-0.5) = -0.5*n + relu-sum
    R3 = dev["M3"] - 0.5 * nvec
    S1 = R1 - 1.5 * C1
    S2 = R2 - 0.5 * C2
    S3 = R3 + 0.5 * C3
    Q1 = dev["P1"] - 3.0 * R1 + 2.25 * C1
    Q2 = dev["P2"] - 1.0 * R2 + 0.25 * C2
    Q3 = dev["P3"] + 1.0 * R3 + 0.25 * C3
    c_ch = np.stack([nvec - C1, C1 - C2, C2 - C3, C3], 1)
    s_ch = a * np.stack([T1 - S1, S1 - S2, S2 - S3, S3], 1)
    q_ch = a * a * np.stack([dev["T2"] - Q1, Q1 - Q2, Q2 - Q3, Q3], 1)

    # ---- model of the reference's sequential-f32 segment_sum ----
    samp = w_full[::8].astype(np.float64)
    bins_s = np.round(np.clip(w_full[::8].astype(np.float32) / f32(a),
                              -2, 1)).astype(np.int64) + 2
    bin_vals = [samp[bins_s == k] for k in range(4)]
    beta_cache = {}

    def beta(kind, k, u):
        key = (kind, k, int(np.log2(u)))
        if key not in beta_cache:
            v = bin_vals[k]
            v = v * v if kind == "q" else v
            sv = v.sum()
            beta_cache[key] = (u * np.round(v / u)).sum() / sv if sv != 0 else 1.0
        return beta_cache[key]

    def replay(kind, k, deltas):
        nz = deltas[deltas != 0]
        scale = np.median(np.abs(nz)) / F if nz.size else 1.0
        Pp = 0.0
        for m in range(NCHUNK):
            ap = abs(Pp)
            if ap == 0.0:
                Pp += deltas[m]
                continue
            u = 2.0 ** (np.floor(np.log2(ap)) - 23)
            if u < 1e-3 * scale:
                Pp += deltas[m]
            else:
                Pp += beta(kind, k, u) * deltas[m]
        return Pp

    c_tot = c_ch.sum(0)
    c_f32 = np.minimum(c_tot, 2.0 ** 24)
    s_f32 = np.array([replay("s", k, s_ch[:, k]) for k in range(4)])
    q_f32 = np.array([replay("q", k, q_ch[:, k]) for k in range(4)])

    L = np.array([-2.0, -1.0, 0.0, 1.0]) * a
    safe = np.maximum(c_f32, 1.0)
    mean = s_f32 / safe
    var = q_f32 / safe - mean * mean
    total_mse = np.where(c_tot > 0, (mean - L) ** 2, 0.0).sum()
    total_var = np.where(c_tot >= 2, var, 0.0).sum()
    loss = total_mse + total_var

    s_ex = s_ch.sum(0)
    q_ex = q_ch.sum(0)
    sum_d2 = (q_ex - 2 * L * s_ex + L * L * c_tot).sum()
    mean_dist = a * dev["A"].sum() / N

    return np.array([loss, total_mse, total_var, sum_d2 / N, mean_dist],
                    np.float32)
